# revision 24
# baseline (speedup 1.0000x reference)
"""Trainium2 Bass kernel for BinarySplitDecoder (binary-tree leaf probabilities).

Contract: kernel(x) takes the FULL input x [65536, 1023] fp32 and returns the
FULL output [65536, 1024] fp32 (leaf probabilities of a depth-10 binary split
tree, level-major node ordering).

Sharding: pure data parallel - batch dim split evenly across 8 NeuronCores.

Per-core kernel (rows_per_core = 8192, memory-bound at ~33.5 MB of fp16 HBM
I/O against a ~420 GB/s 16-engine DMA wall):
  - fp16 I/O: the host converts x to fp16 and upcasts y back, halving HBM
    traffic. Tolerance is 2e-2 relative to absmax; measured end-to-end error
    of the all-fp16 pipeline is ~1.5e-3.
  - Block (bit-reversal) layout: each level writes left-children into the
    first half and right-children into the second half of the next level's
    tile, so every DVE operand has a packed (stride-1) last dim. That avoids
    the ~1.7x strided-write penalty AND qualifies every tensor_tensor for
    the DVE 2x_1p perf mode (0.52 ns/elem/partition instead of 1.04). The
    resulting column order of y is bit-reversed; the host feeds alphas
    pre-permuted per level and un-permutes y columns at the end (cheap numpy
    gathers, not device work).
  - right = cur - left replaces right = cur * (1 - a): no "1 - x" pass.
  - Two passes: levels 0-5 run ONCE for all 8192 rows (partition p owns rows
    p*64..p*64+63) in 12 large DVE ops - the per-op sequencer overhead that
    dominated the small levels amortizes away. Levels 6-9 then run per row-
    chunk, pipelined against the loads of their alphas (xb) and the stores.
  - The level 0-5 alphas (xa, cols 0:63) and level 6-9 alphas (xb, cols
    63:1023) are separate DRAM params so both load fully contiguous.
  - Loads ride the ACT-sequencer HWDGE queue; stores alternate between the
    SP and GPSIMD queues (one store queue caps at ~210 GB/s of descriptor
    issue and becomes the tail; two drain in parallel and keep the 16 DMA
    engines fed together with the load queue).
  - Small leading/trailing chunks shorten the pipeline ramp and the final
    store tail.
"""

import numpy as np

import concourse.bacc as bacc
import concourse.bass as bass
import concourse.mybir as mybir
from concourse.tile import TileContext
from concourse.bass_utils import run_bass_kernel_spmd

TREE_DEPTH = 10
N_NODES = (1 << TREE_DEPTH) - 1  # 1023
N_LEAVES = 1 << TREE_DEPTH  # 1024
N_CORES = 8
P = 128  # SBUF partitions
SPLIT_D = 6  # levels < SPLIT_D run in pass A; levels >= SPLIT_D run in pass B
NA = (1 << SPLIT_D) - 1  # 63 alpha columns consumed by pass A
NB = N_NODES - NA  # 960 alpha columns consumed by pass B
NB1 = 448  # levels 6-8 (64 + 128 + 256 columns)
NB2 = NB - NB1  # level 9 (512 columns)


def _bitrev(j: int, bits: int) -> int:
    r = 0
    for _ in range(bits):
        r = (r << 1) | (j & 1)
        j >>= 1
    return r


def _input_perm() -> np.ndarray:
    """perm[k] = source column of x for device column k (level-major order,
    bit-reversed node index within each level)."""
    perm = np.empty(N_NODES, dtype=np.int64)
    for d in range(TREE_DEPTH):
        base = (1 << d) - 1
        for j in range(1 << d):
            perm[base + j] = base + _bitrev(j, d)
    return perm


def _output_perm() -> np.ndarray:
    """y[:, t] = y_dev[:, outperm[t]] (bit reversal, self-inverse)."""
    return np.array([_bitrev(t, TREE_DEPTH) for t in range(N_LEAVES)], dtype=np.int64)


_IN_PERM = _input_perm()
_OUT_PERM = _output_perm()


def build_nc(rows_per_core: int, G: int = 8,
             lead: tuple = (4, 4), tail: tuple = (4, 2, 2)) -> bass.Bass:
    """Build the per-core Bass program.

    DRAM params (fp16, columns pre-permuted per level on the host):
      xa [rows, 63]   alphas for levels 0-5
      xb [rows, 960]  alphas for levels 6-9
      y  [rows, 1024] leaf probabilities, columns in bit-reversed order
    """
    U = rows_per_core // P  # row-units per partition; partition p owns
    # global rows p*U + u for u in [0, U)
    body = U - sum(lead) - sum(tail)
    assert body > 0 and body % G == 0
    chunks = list(lead) + [G] * (body // G) + list(tail)
    assert sum(chunks) == U
    f16 = mybir.dt.float16

    nc = bacc.Bacc("TRN2", target_bir_lowering=False, debug=False)
    xa = nc.declare_dram_parameter("xa", [rows_per_core, NA], f16, isOutput=False)
    xb1 = nc.declare_dram_parameter("xb1", [rows_per_core, NB1], f16, isOutput=False)
    xb2 = nc.declare_dram_parameter("xb2", [rows_per_core, NB2], f16, isOutput=False)
    y = nc.declare_dram_parameter("y", [rows_per_core, N_LEAVES], f16, isOutput=True)

    xb1_v = xb1.rearrange("(p u) n -> p u n", p=P, u=U)
    xb2_v = xb2.rearrange("(p u) n -> p u n", p=P, u=U)
    y_v = y.rearrange("(p u) m -> p u m", p=P, u=U)

    with TileContext(nc) as tc:
        with (
            tc.tile_pool(name="pre", bufs=1) as prep,
            tc.tile_pool(name="xin", bufs=5) as xp,
            tc.tile_pool(name="out", bufs=4) as outp,
            tc.tile_pool(name="cur", bufs=2) as curp,
        ):
            # ---- pass A: levels 0..5 for all rows, one shot ----
            xat = prep.tile([P, U, NA], f16, tag="xa")
            nc.scalar.dma_start(
                out=xat[:], in_=xa[:, :].rearrange("(p u) n -> p (u n)", p=P, u=U)
            )
            cur = None
            for d in range(SPLIT_D):
                L = 1 << d
                nxt = prep.tile([P, U, 2 * L], f16, tag=f"pre{d % 2}")
                a = xat[:, :, L - 1 : 2 * L - 1]
                left = nxt[:, :, 0:L]
                right = nxt[:, :, L : 2 * L]
                if d == 0:
                    nc.vector.tensor_copy(out=left, in_=a)
                    nc.vector.tensor_scalar(
                        out=right,
                        in0=a,
                        scalar1=-1.0,
                        scalar2=1.0,
                        op0=mybir.AluOpType.mult,
                        op1=mybir.AluOpType.add,
                    )
                else:
                    nc.vector.tensor_mul(out=left, in0=cur, in1=a)
                    nc.vector.tensor_sub(out=right, in0=cur, in1=left)
                cur = nxt
            curA = cur  # [P, U, 64] level-5 probabilities, persists for pass B

            # ---- pass B: levels 6..9, pipelined row chunks ----
            store_q = [nc.sync, nc.gpsimd]
            u0 = 0
            for c, g in enumerate(chunks):
                # Split the alpha load: levels 6-8 land first so compute
                # starts sooner; level 9's alphas stream in while levels 6-8
                # are being computed. Separate DRAM params keep both loads
                # fully contiguous.
                xt = xp.tile([P, g, NB1], f16, tag="x1")
                nc.scalar.dma_start(out=xt[:], in_=xb1_v[:, u0 : u0 + g, :])
                xt2 = xp.tile([P, g, NB2], f16, tag="x2")
                nc.scalar.dma_start(out=xt2[:], in_=xb2_v[:, u0 : u0 + g, :])

                out_t = outp.tile([P, g, N_LEAVES], f16, tag="y")
                cur = curA[:, u0 : u0 + g, :]
                col = 0
                for d in range(SPLIT_D, TREE_DEPTH):
                    L = 1 << d
                    if d == TREE_DEPTH - 1:
                        nxt = out_t
                    else:
                        nxt = curp.tile([P, g, 2 * L], f16, tag=f"cur{d % 2}")
                    if d == TREE_DEPTH - 1:
                        a = xt2[:, :, :]
                    else:
                        a = xt[:, :, col : col + L]
                        col += L
                    left = nxt[:, :, 0:L]
                    right = nxt[:, :, L : 2 * L]
                    nc.vector.tensor_mul(out=left, in0=cur, in1=a)
                    if d == TREE_DEPTH - 1:
                        # Nothing downstream consumes the last level's right
                        # half (it feeds only the store), so GPSIMD can
                        # compute it fully in parallel with DVE moving on.
                        nc.gpsimd.tensor_sub(out=right, in0=cur, in1=left)
                    else:
                        nc.vector.tensor_sub(out=right, in0=cur, in1=left)
                    cur = nxt

                store_q[c % 2].dma_start(out=y_v[:, u0 : u0 + g, :], in_=out_t[:])
                u0 += g

    nc.compile()
    return nc


def _run(x: np.ndarray, **spmd_kwargs):
    """Shard x, run the Bass kernel on all 8 cores, return (y, BassKernelResults)."""
    x = np.asarray(x)
    B = x.shape[0]
    assert B % N_CORES == 0 and x.shape[1] == N_NODES
    rows_per_core = B // N_CORES

    # fp16 + per-level bit-reversed column order (see module docstring).
    x16 = x.astype(np.float16)[:, _IN_PERM]
    xa = np.ascontiguousarray(x16[:, :NA])
    xb1 = np.ascontiguousarray(x16[:, NA : NA + NB1])
    xb2 = np.ascontiguousarray(x16[:, NA + NB1 :])

    nc = build_nc(rows_per_core)
    core_ids = list(range(N_CORES))
    in_maps = [
        {
            "xa": xa[i * rows_per_core : (i + 1) * rows_per_core],
            "xb1": xb1[i * rows_per_core : (i + 1) * rows_per_core],
            "xb2": xb2[i * rows_per_core : (i + 1) * rows_per_core],
        }
        for i in core_ids
    ]
    res = run_bass_kernel_spmd(nc, in_maps, core_ids, **spmd_kwargs)
    y16 = np.concatenate([r["y"] for r in res.results], axis=0)
    out = y16[:, _OUT_PERM].astype(np.float32)
    return out, res


def kernel(x: np.ndarray) -> np.ndarray:
    return _run(x)[0]


# revision 26
# speedup vs baseline: 1.6422x; 1.6422x over previous
"""Trainium2 Bass kernel for BinarySplitDecoder (binary-tree leaf probabilities).

Contract: kernel(x) takes the FULL input x [65536, 1023] fp32 and returns the
FULL output [65536, 1024] fp32 (leaf probabilities of a depth-10 binary split
tree, level-major node ordering).

Sharding: pure data parallel - batch dim split evenly across 8 NeuronCores.

Per-core kernel (rows_per_core = 8192; memory-bound: ~33.5 MB of fp16 HBM
I/O against the ~420-450 GB/s 16-engine DMA wall, so the target is "DMA
saturated end-to-end, DVE hidden under it"):
  - fp16 I/O: the host converts x to fp16 and upcasts y back, halving HBM
    traffic. Tolerance is 2e-2 relative to absmax; measured end-to-end error
    of the all-fp16 pipeline is ~1.5e-3.
  - Block (bit-reversal) layout: each level writes left-children into the
    first half and right-children into the second half of the next level's
    tile, so every DVE operand has a packed (stride-1) last dim. That avoids
    the ~1.7x strided-write penalty AND qualifies every tensor_tensor for
    the DVE 2x_1p perf mode (0.52 ns/elem/partition instead of 1.04). The
    resulting column order of y is bit-reversed; the host feeds alphas
    pre-permuted per level and un-permutes y columns at the end (cheap numpy
    gathers, not device work).
  - right = cur - left replaces right = cur * (1 - a): no "1 - x" pass.
  - Two passes: levels 0-7 run ONCE for all 8192 rows (partition p owns rows
    p*64..p*64+63) in 16 large DVE ops - the per-op sequencer overhead that
    would dominate the small levels amortizes away. Levels 8-9 (7/8 of the
    element work and 3/4 of the input bytes) then run per row-chunk,
    pipelined against their alpha loads and the output stores.
  - Each level group is its own DRAM param (xa0 = levels 0-5, xa6, xa7, xb8,
    xb9) so every DMA is a fully contiguous block; splitting level 8 and 9
    loads also lets a chunk start computing once its level-8 alphas land.
  - Pass-A loads ride the SP (store) queue, which is idle until the first
    chunk finishes; chunk loads ride the ACT-sequencer queue; stores
    alternate between SP and GPSIMD queues (a single store queue caps at
    ~210 GB/s of descriptor issue and becomes the tail).
  - Small leading/trailing chunks shorten the pipeline ramp and the final
    store tail. GPSIMD never runs tensor ops (its software loops are ~10x
    slower and stall the pipeline - measured).
"""

import numpy as np

import concourse.bacc as bacc
import concourse.bass as bass
import concourse.mybir as mybir
from concourse.tile import TileContext
from concourse.bass_utils import run_bass_kernel_spmd

TREE_DEPTH = 10
N_NODES = (1 << TREE_DEPTH) - 1  # 1023
N_LEAVES = 1 << TREE_DEPTH  # 1024
N_CORES = 8
P = 128  # SBUF partitions
SPLIT_D = 8  # levels < SPLIT_D run in pass A; levels >= SPLIT_D run in pass B
NA0 = 63  # alpha columns for levels 0-5
NA6 = 64  # level 6
NA7 = 128  # level 7
NB8 = 256  # level 8
NB9 = 512  # level 9


def _bitrev(j: int, bits: int) -> int:
    r = 0
    for _ in range(bits):
        r = (r << 1) | (j & 1)
        j >>= 1
    return r


def _input_perm() -> np.ndarray:
    """perm[k] = source column of x for device column k (level-major order,
    bit-reversed node index within each level)."""
    perm = np.empty(N_NODES, dtype=np.int64)
    for d in range(TREE_DEPTH):
        base = (1 << d) - 1
        for j in range(1 << d):
            perm[base + j] = base + _bitrev(j, d)
    return perm


def _output_perm() -> np.ndarray:
    """y[:, t] = y_dev[:, outperm[t]] (bit reversal, self-inverse)."""
    return np.array([_bitrev(t, TREE_DEPTH) for t in range(N_LEAVES)], dtype=np.int64)


_IN_PERM = _input_perm()
_OUT_PERM = _output_perm()


def build_nc(rows_per_core: int, G: int = 8,
             lead: tuple = (4, 4), tail: tuple = (4, 2, 2)) -> bass.Bass:
    """Build the per-core Bass program (see module docstring)."""
    U = rows_per_core // P  # row-units per partition; partition p owns
    # global rows p*U + u for u in [0, U)
    body = U - sum(lead) - sum(tail)
    assert body > 0 and body % G == 0
    chunks = list(lead) + [G] * (body // G) + list(tail)
    assert sum(chunks) == U
    f16 = mybir.dt.float16

    nc = bacc.Bacc("TRN2", target_bir_lowering=False, debug=False)
    xa0 = nc.declare_dram_parameter("xa0", [rows_per_core, NA0], f16, isOutput=False)
    xa6 = nc.declare_dram_parameter("xa6", [rows_per_core, NA6], f16, isOutput=False)
    xa7 = nc.declare_dram_parameter("xa7", [rows_per_core, NA7], f16, isOutput=False)
    xb8 = nc.declare_dram_parameter("xb8", [rows_per_core, NB8], f16, isOutput=False)
    xb9 = nc.declare_dram_parameter("xb9", [rows_per_core, NB9], f16, isOutput=False)
    y = nc.declare_dram_parameter("y", [rows_per_core, N_LEAVES], f16, isOutput=True)

    def full_view(t, n):
        return t[:, :].rearrange("(p u) n -> p (u n)", p=P, u=U)

    xb8_v = xb8.rearrange("(p u) n -> p u n", p=P, u=U)
    xb9_v = xb9.rearrange("(p u) n -> p u n", p=P, u=U)
    y_v = y.rearrange("(p u) m -> p u m", p=P, u=U)

    with TileContext(nc) as tc:
        with (
            tc.tile_pool(name="pre", bufs=1) as prep,
            tc.tile_pool(name="xin", bufs=4) as xp,
            tc.tile_pool(name="out", bufs=3) as outp,
            tc.tile_pool(name="cur", bufs=2) as curp,
        ):
            # ---- pass A: levels 0..7 for all rows, one shot ----
            xa0t = prep.tile([P, U, NA0], f16, tag="xa0")
            nc.sync.dma_start(out=xa0t[:], in_=full_view(xa0, NA0))
            xa6t = prep.tile([P, U, NA6], f16, tag="xa6")
            nc.sync.dma_start(out=xa6t[:], in_=full_view(xa6, NA6))
            xa7t = prep.tile([P, U, NA7], f16, tag="xa7")
            nc.sync.dma_start(out=xa7t[:], in_=full_view(xa7, NA7))

            cur = None
            for d in range(SPLIT_D):
                L = 1 << d
                nxt = prep.tile([P, U, 2 * L], f16, tag=f"pre{d % 2}")
                if d < 6:
                    a = xa0t[:, :, L - 1 : 2 * L - 1]
                elif d == 6:
                    a = xa6t[:, :, :]
                else:
                    a = xa7t[:, :, :]
                left = nxt[:, :, 0:L]
                right = nxt[:, :, L : 2 * L]
                if d == 0:
                    nc.vector.tensor_copy(out=left, in_=a)
                    nc.vector.tensor_scalar(
                        out=right,
                        in0=a,
                        scalar1=-1.0,
                        scalar2=1.0,
                        op0=mybir.AluOpType.mult,
                        op1=mybir.AluOpType.add,
                    )
                else:
                    nc.vector.tensor_mul(out=left, in0=cur, in1=a)
                    nc.vector.tensor_sub(out=right, in0=cur, in1=left)
                cur = nxt
            curA = cur  # [P, U, 256] level-7 probabilities, persists for pass B

            # ---- pass B: levels 8..9, pipelined row chunks ----
            store_q = [nc.sync, nc.gpsimd]
            u0 = 0
            for c, g in enumerate(chunks):
                x8t = xp.tile([P, g, NB8], f16, tag="x8")
                nc.scalar.dma_start(out=x8t[:], in_=xb8_v[:, u0 : u0 + g, :])
                x9t = xp.tile([P, g, NB9], f16, tag="x9")
                nc.scalar.dma_start(out=x9t[:], in_=xb9_v[:, u0 : u0 + g, :])

                cur8 = curp.tile([P, g, 2 * NB8], f16, tag="cur8")
                c7 = curA[:, u0 : u0 + g, :]
                left = cur8[:, :, 0:NB8]
                right = cur8[:, :, NB8 : 2 * NB8]
                nc.vector.tensor_mul(out=left, in0=c7, in1=x8t[:])
                nc.vector.tensor_sub(out=right, in0=c7, in1=left)

                out_t = outp.tile([P, g, N_LEAVES], f16, tag="y")
                left = out_t[:, :, 0:NB9]
                right = out_t[:, :, NB9:N_LEAVES]
                nc.vector.tensor_mul(out=left, in0=cur8[:], in1=x9t[:])
                nc.vector.tensor_sub(out=right, in0=cur8[:], in1=left)

                store_q[c % 2].dma_start(out=y_v[:, u0 : u0 + g, :], in_=out_t[:])
                u0 += g

    nc.compile()
    return nc


def _run(x: np.ndarray, **spmd_kwargs):
    """Shard x, run the Bass kernel on all 8 cores, return (y, BassKernelResults)."""
    x = np.asarray(x)
    B = x.shape[0]
    assert B % N_CORES == 0 and x.shape[1] == N_NODES
    rows_per_core = B // N_CORES

    # fp16 + per-level bit-reversed column order (see module docstring).
    x16 = x.astype(np.float16)[:, _IN_PERM]
    splits = np.cumsum([NA0, NA6, NA7, NB8])
    parts = np.split(x16, splits, axis=1)
    names = ["xa0", "xa6", "xa7", "xb8", "xb9"]

    nc = build_nc(rows_per_core)
    core_ids = list(range(N_CORES))
    in_maps = [
        {
            nm: np.ascontiguousarray(p[i * rows_per_core : (i + 1) * rows_per_core])
            for nm, p in zip(names, parts)
        }
        for i in core_ids
    ]
    res = run_bass_kernel_spmd(nc, in_maps, core_ids, **spmd_kwargs)
    y16 = np.concatenate([r["y"] for r in res.results], axis=0)
    out = y16[:, _OUT_PERM].astype(np.float32)
    return out, res


def kernel(x: np.ndarray) -> np.ndarray:
    return _run(x)[0]
